# revision 42
# baseline (speedup 1.0000x reference)
"""CSPNGenerate Trainium2 kernel, v2 (tap-band conv + PE fold).

Per core (8 cores = batch b x row-half): two 88-row strips (A/B) live in
partition halves 0:64 / 64:128 (64 channels each).

Conv restructure vs v1: instead of 9 taps x M=9 matmuls over 2-row windows,
each input row is streamed once per horizontal tap dj (3 passes) with M=27
"tap bands" (3 vertical taps di x 9 outputs: 8 channels + channel-sum row T).
PSUM col group c in 0..3 holds input row 4j-1+c of a quad; bands land at
partitions 32c + 9di + m. One 128-row ACT evacuates the quad (+bias/3 per
band) to an SBUF tile E_j. A K=128 selector matmul ("fold") then sums the 3
row-aligned bands per output row: out rows packed M=18 (col 2m+rp) per col
group q=(strip,pair), giving ys/aas/stage tiles with 72/128-row density.

Downstream (S = sum|y|, R = 1/S via ACT Reciprocal, PE broadcast of R,
DVE multiply to staged planes, plane 4 = -T*R with host adding 1) mirrors v1
but at the new packing. abs runs on DVE (tensor_scalar abs_max).
"""

import sys

if "/opt/trn_rl_repo" not in sys.path:
    sys.path.insert(0, "/opt/trn_rl_repo")

import numpy as np
import concourse.bass as bass
import concourse.mybir as mybir
from concourse.tile import TileContext
from concourse.vector_clock import ScopedClock, VectorClock


# ---- toolchain workarounds (drain-wait split, per-instruction sync-wait
# limit, optional NTFF profiling shim) ----
def _drain_and_barrier_split(self, tick_clock, wait_clock):
    gclock = tick_clock.global_clock
    nprocs = len(gclock)
    for proc in range(nprocs):
        tick = gclock[proc]
        if tick <= 0:
            continue
        vc = VectorClock([0] * nprocs)
        vc.require_at_least(proc, tick)
        nop = self.nc.sync.nop(nofuse=True, hint="drain_split_wait")
        wait_clock.add_sem_waits(nop.ins, ScopedClock({None: vc}))

    self.nc.sync.drain()

    self.nc.all_engine_barrier()
    assert self.sems is not None
    popped = self.nc._tile_sem_poison_stack.pop()
    assert popped is self._sem_poison
    self.nc.clear_and_free_semaphores(list(self.sems.allocated().values()))
    self.nc.all_engine_barrier()


def install():
    TileContext._drain_and_barrier = _drain_and_barrier_split
    install_wait_split()


_MAX_WAITS = 1


def _split_waits_json(bir: bytes) -> bytes:
    """Walrus rejects instructions carrying more than one sync-wait command.
    Move excess waits onto same-engine NoOps inserted before the ins."""
    import orjson

    m = orjson.loads(bir)
    for func in m.get("functions", []):
        for block in func.get("blocks", []):
            out = []
            changed = False
            for inst in block["instructions"]:
                si = inst.get("sync_info") or {}
                waits = si.get("on_wait") or []
                if len(waits) > _MAX_WAITS:
                    keep = waits[-_MAX_WAITS:]
                    extra = waits[:-_MAX_WAITS]
                    for k, w in enumerate(extra):
                        out.append(
                            {
                                "debug": inst.get("debug", 0),
                                "engine": inst["engine"],
                                "ins": [],
                                "name": f"{inst['name']}-wsplit{k}",
                                "opcode": "NoOp",
                                "outs": [],
                                "sync_info": {"on_update": [], "on_wait": [w]},
                                "text_hint": "wait_split",
                            }
                        )
                    si["on_wait"] = keep
                    inst["sync_info"] = si
                    changed = True
                out.append(inst)
            if changed:
                block["instructions"] = out
    return orjson.dumps(m)


def install_wait_split():
    import concourse.bass as _bass

    if getattr(_bass.Bass, "_wait_split_installed", False):
        return
    orig = _bass.Bass.to_json_bytes

    def to_json_bytes(self):
        return _split_waits_json(orig(self))

    _bass.Bass.to_json_bytes = to_json_bytes
    _bass.Bass._wait_split_installed = True


def install_ntff_shim():
    import sys
    import types

    if "antenv.axon_hooks" in sys.modules:
        return
    mod = types.ModuleType("antenv.axon_hooks")
    state = {"hook": None}

    def set_axon_ntff_profile_hook(h):
        state["hook"] = h

    def get_axon_ntff_profile_hook():
        return state["hook"]

    mod.set_axon_ntff_profile_hook = set_axon_ntff_profile_hook
    mod.get_axon_ntff_profile_hook = get_axon_ntff_profile_hook
    sys.modules["antenv.axon_hooks"] = mod

    try:
        from trn_agent_boot.trn_boot import _ntff_profile_via_ctypes

        hook = _ntff_profile_via_ctypes("/opt/axon/libaxon_pjrt.so")
        if hook is not None:
            set_axon_ntff_profile_hook(hook)
    except Exception as e:
        print(f"ntff shim: hook install failed: {e}")

    from concourse import bass_utils

    bass_utils.upload_artifacts = lambda tmpdir: tmpdir


# geometry
B, C, H, W, K = 4, 64, 352, 1216, 3
HALF = 176  # rows per core
STRIP = 88  # rows per strip (2 strips per core in partition halves)
WP = W + 2  # padded width
BLK_ROWS = 16  # input rows per slab block (4 quads)
NBLK = 6  # blocks per strip (96 rows: local input rows -1..94)
NQ = 23  # conv quads per strip (input rows -1..90)
NFQ = 22  # fold quads per strip (out rows 0..87)
XCH = [(0, 512), (512, 512), (1024, 192)]
NCH = len(XCH)
MROW = [0, 1, 2, 3, 8, 4, 5, 6, 7]  # T-row m for output plane p

F32 = mybir.dt.float32
F16 = mybir.dt.float16


def act_reciprocal(nc, out, in_):
    """ACT Reciprocal, bypassing the bass advisory accuracy guard (our
    tolerance is 2e-2; the r8 output is fp16 anyway)."""
    eng = nc.scalar
    inputs = [eng.lower_ap(in_)]
    for arg in (0.0, 1.0, 0.0):  # bias, scale, alpha
        inputs.append(mybir.ImmediateValue(dtype=mybir.dt.float32, value=arg))
    return eng.add_instruction(
        mybir.InstActivation(
            name=nc.get_next_instruction_name(),
            func=mybir.ActivationFunctionType.Reciprocal,
            ins=inputs,
            outs=[eng.lower_ap(out)],
        )
    )


def build_nc():
    nc = bass.Bass()
    slab = nc.dram_tensor(
        "slab", [128, NBLK, BLK_ROWS * WP], F16, kind="ExternalInput"
    )
    w27 = nc.dram_tensor("w27", [128, 162], F16, kind="ExternalInput")
    fsel = nc.dram_tensor("fsel", [128, 54], F16, kind="ExternalInput")
    rbsel = nc.dram_tensor("rbsel", [128, 288], F16, kind="ExternalInput")
    ssel = nc.dram_tensor("ssel", [128, 8], F16, kind="ExternalInput")
    bias = nc.dram_tensor("bias", [128, 1], F32, kind="ExternalInput")
    out = nc.dram_tensor("out", [NFQ // 2, 128, 2 * W], F16, kind="ExternalOutput")

    with TileContext(nc) as tc:
        with (
            tc.tile_pool(name="consts", bufs=1) as cpool,
            tc.tile_pool(name="slabp", bufs=2) as slabp,
            tc.tile_pool(name="epool", bufs=16) as epool,
            tc.tile_pool(name="ysp", bufs=14) as ysp,
            tc.tile_pool(name="aasp", bufs=5) as aasp,
            tc.tile_pool(name="r8p", bufs=4) as r8p,
            tc.tile_pool(name="r32p", bufs=2) as r32p,
            tc.tile_pool(name="stagep", bufs=2) as stagep,
            tc.tile_pool(name="pconv", bufs=3, space="PSUM") as pconv,
            tc.tile_pool(name="pfold", bufs=2, space="PSUM") as pfold,
            tc.tile_pool(name="ps_s", bufs=2, space="PSUM") as ps_s,
            tc.tile_pool(name="ps_rbc", bufs=1, space="PSUM") as ps_rbc,
        ):
            w27t = cpool.tile([128, 162], F16, name="w27t")
            fselt = cpool.tile([128, 54], F16, name="fselt")
            rbselt = cpool.tile([128, 288], F16, name="rbselt")
            sselt = cpool.tile([128, 8], F16, name="sselt")
            biast = cpool.tile([128, 1], F32, name="biast")

            # first slab block split: cols for chunk 0 first, then the rest
            st0 = slabp.tile([128, BLK_ROWS * WP], F16, name="st")
            st0v = st0[:].rearrange("p (r w) -> p r w", r=BLK_ROWS, w=WP)
            sl0v = slab[:, 0, :].rearrange("p (r w) -> p r w", r=BLK_ROWS, w=WP)
            # first the weights + the rows for the first two quads, so the
            # conv starts ~4us in; trailing rows and other consts follow
            nc.sync.dma_start(w27t[:], w27[:])
            nc.sync.dma_start(st0v[:, 0:5, :], sl0v[:, 0:5, :])
            nc.gpsimd.dma_start(biast[:], bias[:])
            nc.gpsimd.dma_start(st0v[:, 5:BLK_ROWS, :], sl0v[:, 5:BLK_ROWS, :])
            nc.gpsimd.dma_start(fselt[:], fsel[:])
            nc.gpsimd.dma_start(rbselt[:], rbsel[:])
            nc.gpsimd.dma_start(sselt[:], ssel[:])

            # one-time PSUM inits (banks rotate within fixed pools):
            # conv + fold banks to 0 (pad rows must read as 0);
            # S banks to 1.0 (reciprocal must not see garbage).
            for pool, nm, cnt in ((pconv, "cv", 3), (pfold, "pf", 2), (ps_rbc, "rbc", 1)):
                for _ in range(cnt):
                    tz = pool.tile([128, 512], F32, name=nm)
                    nc.vector.memset(tz[:], 0.0)
            for _ in range(2):
                tz = ps_s.tile([128, 512], F32, name="s8")
                nc.vector.memset(tz[:], 1.0)

            # ---- software pipeline state ----
            etiles = {}  # (s, j, ch) -> E tile
            recip_batch = {"tile": None, "members": [], "count": 0}
            post_recip = []  # fold-units waiting on their batch reciprocal
            pending_smm = []  # fold-units whose S matmul is deferred 1 unit
            stage_tiles = {}  # (group, ) -> stage tile

            def do_recip_flush():
                b = recip_batch
                if not b["members"]:
                    return
                r8s = [r8p.tile([128, 512], F16, name="r8") for _ in range(2)]
                with nc.allow_low_precision(reason="fp16 normalize"):
                    act_reciprocal(nc, r8s[0][:, 0:512], b["tiles"][0][:, 0:512])
                    if b["tiles"][1] is not None:
                        act_reciprocal(nc, r8s[1][:, 0:512], b["tiles"][1][:, 0:512])
                for u, fu in enumerate(b["members"]):
                    fu["r8"] = r8s[u // 4]
                    fu["u"] = u % 4
                b["tiles"] = [None, None]
                b["members"] = []

            def do_smm(fu):
                b = recip_batch
                u = len(b["members"])
                if u % 4 == 0:
                    b.setdefault("tiles", [None, None])[u // 4] = ps_s.tile(
                        [128, 512], F32, name="s8"
                    )
                n = fu["n"]
                nc.tensor.matmul(
                    out=b["tiles"][u // 4][32 * (u % 4) : 32 * (u % 4) + 8, 0:n],
                    lhsT=sselt[0:114, 0:8],
                    rhs=fu["aas"][0:114, 0:n],
                    start=True,
                    stop=True,
                    tile_position=(0, 32 * (u % 4)),
                    skip_group_check=True,
                )
                b["members"].append(fu)
                if len(b["members"]) == 8:
                    do_recip_flush()

            def do_post(fu):
                # rbc broadcast matmuls + normalize multiply + stage DMA
                n, x0, ch, fj = fu["n"], fu["x0"], fu["ch"], fu["fj"]
                u = fu["u"]
                rbc = ps_rbc.tile([128, 512], F32, name="rbc")
                for q in range(4):
                    bidx = 4 * u + q
                    nc.tensor.matmul(
                        out=rbc[32 * q : 32 * q + 18, 0:n],
                        lhsT=rbselt[:, 18 * bidx : 18 * bidx + 18],
                        rhs=fu["r8"][:, 0:n],
                        start=True,
                        stop=True,
                        tile_position=(0, 32 * q),
                        skip_group_check=True,
                    )
                g, k2 = fj // 2, fj % 2
                if (g, ) not in stage_tiles:
                    stage_tiles[(g, )] = stagep.tile(
                        [128, 2 * W], F16, name="stage"
                    )
                stage = stage_tiles[(g, )]
                sv = stage[:].rearrange("p (k x) -> p k x", k=2, x=W)
                with nc.allow_low_precision(reason="fp16 output staging"):
                    nc.vector.tensor_mul(
                        sv[0:114, k2, x0 : x0 + n],
                        fu["ys"][0:114, 0:n],
                        rbc[0:114, 0:n],
                    )
                if k2 == 1 and ch == NCH - 1:
                    # group complete: one dense raw dump (host de-interleaves)
                    nc.sync.dma_start(out[g], stage[:])
                    del stage_tiles[(g, )]

            def drain_posts(limit):
                k = 0
                while post_recip and "r8" in post_recip[0] and k < limit:
                    do_post(post_recip.pop(0))
                    k += 1

            def do_fold(fj, ch, n, x0):
                # fold quad fj: K=128 untiled matmuls on per-strip E tiles
                # (bands at 32c + 9di + m); pair0 = pattern A on E_j,
                # pair1 = pattern B on E_j + pattern C on E_{j+1}
                pf = pfold.tile([128, 512], F32, name="pf")
                es = [(etiles[(s, fj, ch)], etiles[(s, fj + 1, ch)]) for s in range(2)]
                for s in range(2):
                    nc.tensor.matmul(
                        out=pf[64 * s : 64 * s + 18, 0:n],
                        lhsT=fselt[:, 0:18],
                        rhs=es[s][0][:, 0:n],
                        start=True,
                        stop=True,
                        tile_position=(0, 64 * s),
                        skip_group_check=True,
                    )
                for s in range(2):
                    nc.tensor.matmul(
                        out=pf[64 * s + 32 : 64 * s + 50, 0:n],
                        lhsT=fselt[:, 18:36],
                        rhs=es[s][0][:, 0:n],
                        start=True,
                        stop=False,
                        tile_position=(0, 64 * s + 32),
                        skip_group_check=True,
                    )
                for s in range(2):
                    nc.tensor.matmul(
                        out=pf[64 * s + 32 : 64 * s + 50, 0:n],
                        lhsT=fselt[:, 36:54],
                        rhs=es[s][1][:, 0:n],
                        start=False,
                        stop=True,
                        tile_position=(0, 64 * s + 32),
                        skip_group_check=True,
                    )
                if ch == NCH - 1:
                    # E tiles of quad fj fully consumed
                    for s in range(2):
                        for c2 in range(NCH):
                            etiles.pop((s, fj, c2), None)
                import os
                ys = ysp.tile([128, 512], F16, name="ys")
                aas = aasp.tile([128, 512], F16, name="aas")
                _v = os.environ.get("CSPN_VARIANT", "")
                with nc.allow_low_precision(reason="fp16 y staging"):
                    nc.vector.tensor_copy(ys[0:114, 0:n], pf[0:114, 0:n])
                    # abs on DVE from the fp16 SBUF copy: max(-ys, ys)
                    nc.vector.scalar_tensor_tensor(
                        out=aas[0:114, 0:n],
                        in0=ys[0:114, 0:n],
                        scalar=-1.0,
                        in1=ys[0:114, 0:n],
                        op0=mybir.AluOpType.mult,
                        op1=mybir.AluOpType.max,
                    )
                fu = {"n": n, "x0": x0, "ch": ch, "fj": fj, "ys": ys, "aas": aas}
                # defer this funit's S matmul one unit so the PE never waits
                # on the DVE abs chain
                pending_smm.append(fu)
                if len(pending_smm) > 2:
                    fu2 = pending_smm.pop(0)
                    do_smm(fu2)
                    post_recip.append(fu2)
                drain_posts(1)

            # ---- main loop: conv quads ----
            st = st0
            st_next = None
            for j in range(NQ):
                blk = (4 * j) // BLK_ROWS
                if j % 4 == 0 and blk + 1 < NBLK:
                    st_next = slabp.tile([128, BLK_ROWS * WP], F16, name="st")
                    nc.gpsimd.dma_start(st_next[:], slab[:, blk + 1, :])
                stv = st[:].rearrange("p (r w) -> p r w", r=BLK_ROWS, w=WP)
                for ch, (x0, n) in enumerate(XCH):
                    cbs = [
                        pconv.tile([128, 512], F32, name="cv"),
                        pconv.tile([128, 512], F32, name="cv"),
                    ]
                    # uniform un-row-tiled 128x32 mode everywhere (mode
                    # switches drain the PE); c innermost so consecutive MMs
                    # land on different col tiles; the wrong strip's rows are
                    # zeroed in the weights. Last quad: rows 89,90 are
                    # garbage (never read by any fold) - skip their matmuls.
                    cmax = 2 if j == NQ - 1 else 4
                    for dj in range(3):
                        for s in range(2):
                            for c in range(cmax):
                                rib = (4 * j - 1 + c) - BLK_ROWS * blk + 1
                                nc.tensor.matmul(
                                    out=cbs[s][32 * c : 32 * c + 27, 0:n],
                                    lhsT=w27t[
                                        :,
                                        27 * (3 * s + dj) : 27 * (3 * s + dj) + 27,
                                    ],
                                    rhs=stv[
                                        :,
                                        rib,
                                        x0 + dj : x0 + dj + n,
                                    ],
                                    start=(dj == 0),
                                    stop=(dj == 2),
                                    tile_position=(0, 32 * c),
                                    skip_group_check=True,
                                )
                    # evacuate conv psum (+bias/3 on each band) to E tiles
                    with nc.allow_low_precision(reason="fp16 band staging"):
                        for s in range(2):
                            et = epool.tile([128, 512], F16, name="et")
                            nc.scalar.activation(
                                et[:, 0:n],
                                cbs[s][:, 0:n],
                                mybir.ActivationFunctionType.Identity,
                                bias=biast[:, 0:1],
                                scale=1.0,
                            )
                            etiles[(s, j, ch)] = et
                    # fold of quad j-2 after the evac emission (scalar queue
                    # gets the evacs first); its E tiles are 3+ units old
                    if j >= 2:
                        do_fold(j - 2, ch, n, x0)
                if (4 * (j + 1)) // BLK_ROWS != blk and blk + 1 < NBLK:
                    st = st_next
            # epilogue: last fold quad + drain
            for ch, (x0, n) in enumerate(XCH):
                do_fold(NFQ - 1, ch, n, x0)
            while pending_smm:
                fu2 = pending_smm.pop(0)
                do_smm(fu2)
                post_recip.append(fu2)
            do_recip_flush()
            drain_posts(len(post_recip) + 1)
    return nc


def make_consts(conv_w, gamma, beta, mean, var):
    eps = 1e-5
    s = gamma.astype(np.float64) / np.sqrt(var.astype(np.float64) + eps)
    bt = beta.astype(np.float64) - mean.astype(np.float64) * s
    wp = conv_w.astype(np.float64) * s[:, None, None, None]  # [8, 64, 3, 3]
    bt9 = np.concatenate([bt, [bt.sum()]])  # [9]

    # w27 [128, 162]: col block (s, dj) at 27*(3s+dj); rows 64s..64s+64 hold
    # the weights (cols 9*di + m within block), the other strip's rows are 0
    w27 = np.zeros((128, 162), np.float32)
    for s in range(2):
        for dj in range(3):
            for di in range(3):
                blk = wp[:, :, di, dj]  # [oc, c]
                col = 27 * (3 * s + dj) + 9 * di
                w27[64 * s : 64 * s + 64, col : col + 8] = blk.T
                w27[64 * s : 64 * s + 64, col + 8] = blk.sum(axis=0)

    # bias [128, 1]: rows 32c + 9di + m -> bt9[m]/3
    bias = np.zeros((128, 1), np.float32)
    for c in range(4):
        for di in range(3):
            bias[32 * c + 9 * di : 32 * c + 9 * di + 9, 0] = bt9 / 3.0

    # fold selectors [128, 36]: patterns P1, P2 (M=18, col 2m+rp),
    # identical in both partition halves. E-tile local rows: first band at
    # 9di+m base 0 (ev: c0 / od: c2), second at 32+9di+m (ev: c1 / od: c3).
    # P1: rp0 <- (band0, di0) rows m, (band1, di1) rows 41+m;
    #     rp1 <- (band1, di0) rows 32+m.
    # P2: rp0 <- (band0, di2) rows 18+m;
    #     rp1 <- (band0, di1) rows 9+m, (band1, di2) rows 50+m.
    fsel = np.zeros((128, 54), np.float32)

    def fs(pat, rp, m, c, di):
        fsel[32 * c + 9 * di + m, 18 * pat + 2 * m + rp] = 1.0

    for m in range(9):
        for di in range(3):
            fs(0, 0, m, di, di)
            fs(0, 1, m, 1 + di, di)
        fs(1, 0, m, 2, 0)
        fs(1, 0, m, 3, 1)
        fs(1, 1, m, 3, 0)
        fs(2, 0, m, 0, 2)
        fs(2, 1, m, 0, 1)
        fs(2, 1, m, 1, 2)

    # ssel [128, 8]: row 32q + 2m + rp -> col 2q + rp (m = 0..7)
    ssel = np.zeros((128, 8), np.float32)
    for q in range(4):
        for rp in range(2):
            for m in range(8):
                ssel[32 * q + 2 * m + rp, 2 * q + rp] = 1.0

    # rbsel [128, 288]: 16 patterns (u batch slot, q): col 18*(4u+q) + 2m+rp,
    # nonzero at r8{A,B} row 32u + 2*(q%2) + rp; -1 for m=8 (T row)
    rbsel = np.zeros((128, 288), np.float32)
    for u in range(4):
        for q in range(4):
            for rp in range(2):
                for m in range(9):
                    rbsel[32 * u + 2 * q + rp, 18 * (4 * u + q) + 2 * m + rp] = (
                        -1.0 if m == 8 else 1.0
                    )

    return (
        w27.astype(np.float16),
        fsel.astype(np.float16),
        rbsel.astype(np.float16),
        ssel.astype(np.float16),
        bias,
    )


TRACE = False
LAST_EXEC_NS = None


def kernel(feature, conv_w, gamma, beta, mean, var, kernel_size):
    global LAST_EXEC_NS
    install()
    if TRACE:
        install_ntff_shim()

    from concourse.bass_utils import run_bass_kernel_spmd

    feature = np.asarray(feature, np.float32)
    conv_w = np.asarray(conv_w, np.float32)
    gamma = np.asarray(gamma, np.float32)
    beta = np.asarray(beta, np.float32)
    mean = np.asarray(mean, np.float32)
    var = np.asarray(var, np.float32)

    w27, fsel, rbsel, ssel, bias = make_consts(conv_w, gamma, beta, mean, var)

    # padded feature with extra tail rows for the slab block overhang
    fpad = np.zeros((B, C, H + 10, WP), np.float32)
    fpad[:, :, 1 : H + 1, 1 : W + 1] = feature
    fpad16 = fpad.astype(np.float16)

    in_maps = []
    for core in range(8):
        b, half = core // 2, core % 2
        h0 = half * HALF
        # strip A: core rows 0..87  -> fpad rows h0 + [0, 96)   (block k: 16k)
        # strip B: core rows 88..175 -> fpad rows h0+88 + [0, 96)
        slab2 = np.empty((128, NBLK, BLK_ROWS * WP), np.float16)
        for k in range(NBLK):
            r0 = h0 + BLK_ROWS * k
            slab2[0:64, k, :] = fpad16[b, :, r0 : r0 + BLK_ROWS, :].reshape(C, -1)
            slab2[64:128, k, :] = fpad16[
                b, :, r0 + STRIP : r0 + STRIP + BLK_ROWS, :
            ].reshape(C, -1)
        in_maps.append(
            {
                "slab": slab2,
                "w27": w27,
                "fsel": fsel,
                "rbsel": rbsel,
                "ssel": ssel,
                "bias": bias,
            }
        )

    nc = build_nc()
    res = run_bass_kernel_spmd(nc, in_maps, core_ids=list(range(8)), trace=TRACE)
    LAST_EXEC_NS = res.exec_time_ns

    out_full = np.zeros((B, 9, H + 2, WP), np.float32)
    for core in range(8):
        b, half = core // 2, core % 2
        h0 = half * HALF
        raw = res.results[core]["out"]  # [11, 128, 2*W] fp16
        # row 32q + 2m + rp (q = 2s + pp), free (k, x);
        # out row y = s*88 + 8g + 4k + 2pp + rp
        v = raw.reshape(NFQ // 2, 2, 2, 32, 2, W)  # g s pp v k x
        for p in range(9):
            i, j = p // 3, p % 3
            m = MROW[p]
            sub = v[:, :, :, 2 * m : 2 * m + 2, :, :]  # g s pp rp k x
            plane = (
                sub.transpose(1, 0, 4, 2, 3, 5)  # s g k pp rp x
                .reshape(HALF, W)
                .astype(np.float32)
            )
            if p == 4:
                plane = 1.0 + plane
            out_full[b, p, h0 + i : h0 + HALF + i, j : j + W] = plane
    return out_full


# revision 43
# speedup vs baseline: 1.0062x; 1.0062x over previous
"""CSPNGenerate Trainium2 kernel, v2 (tap-band conv + PE fold).

Per core (8 cores = batch b x row-half): two 88-row strips (A/B) live in
partition halves 0:64 / 64:128 (64 channels each).

Conv restructure vs v1: instead of 9 taps x M=9 matmuls over 2-row windows,
each input row is streamed once per horizontal tap dj (3 passes) with M=27
"tap bands" (3 vertical taps di x 9 outputs: 8 channels + channel-sum row T).
PSUM col group c in 0..3 holds input row 4j-1+c of a quad; bands land at
partitions 32c + 9di + m. One 128-row ACT evacuates the quad (+bias/3 per
band) to an SBUF tile E_j. A K=128 selector matmul ("fold") then sums the 3
row-aligned bands per output row: out rows packed M=18 (col 2m+rp) per col
group q=(strip,pair), giving ys/aas/stage tiles with 72/128-row density.

Downstream (S = sum|y|, R = 1/S via ACT Reciprocal, PE broadcast of R,
DVE multiply to staged planes, plane 4 = -T*R with host adding 1) mirrors v1
but at the new packing. abs runs on DVE (tensor_scalar abs_max).
"""

import sys

if "/opt/trn_rl_repo" not in sys.path:
    sys.path.insert(0, "/opt/trn_rl_repo")

import numpy as np
import concourse.bass as bass
import concourse.mybir as mybir
from concourse.tile import TileContext
from concourse.vector_clock import ScopedClock, VectorClock


# ---- toolchain workarounds (drain-wait split, per-instruction sync-wait
# limit, optional NTFF profiling shim) ----
def _drain_and_barrier_split(self, tick_clock, wait_clock):
    gclock = tick_clock.global_clock
    nprocs = len(gclock)
    for proc in range(nprocs):
        tick = gclock[proc]
        if tick <= 0:
            continue
        vc = VectorClock([0] * nprocs)
        vc.require_at_least(proc, tick)
        nop = self.nc.sync.nop(nofuse=True, hint="drain_split_wait")
        wait_clock.add_sem_waits(nop.ins, ScopedClock({None: vc}))

    self.nc.sync.drain()

    self.nc.all_engine_barrier()
    assert self.sems is not None
    popped = self.nc._tile_sem_poison_stack.pop()
    assert popped is self._sem_poison
    self.nc.clear_and_free_semaphores(list(self.sems.allocated().values()))
    self.nc.all_engine_barrier()


def install():
    TileContext._drain_and_barrier = _drain_and_barrier_split
    install_wait_split()


_MAX_WAITS = 1


def _split_waits_json(bir: bytes) -> bytes:
    """Walrus rejects instructions carrying more than one sync-wait command.
    Move excess waits onto same-engine NoOps inserted before the ins."""
    import orjson

    m = orjson.loads(bir)
    for func in m.get("functions", []):
        for block in func.get("blocks", []):
            out = []
            changed = False
            for inst in block["instructions"]:
                si = inst.get("sync_info") or {}
                waits = si.get("on_wait") or []
                if len(waits) > _MAX_WAITS:
                    keep = waits[-_MAX_WAITS:]
                    extra = waits[:-_MAX_WAITS]
                    for k, w in enumerate(extra):
                        out.append(
                            {
                                "debug": inst.get("debug", 0),
                                "engine": inst["engine"],
                                "ins": [],
                                "name": f"{inst['name']}-wsplit{k}",
                                "opcode": "NoOp",
                                "outs": [],
                                "sync_info": {"on_update": [], "on_wait": [w]},
                                "text_hint": "wait_split",
                            }
                        )
                    si["on_wait"] = keep
                    inst["sync_info"] = si
                    changed = True
                out.append(inst)
            if changed:
                block["instructions"] = out
    return orjson.dumps(m)


def install_wait_split():
    import concourse.bass as _bass

    if getattr(_bass.Bass, "_wait_split_installed", False):
        return
    orig = _bass.Bass.to_json_bytes

    def to_json_bytes(self):
        return _split_waits_json(orig(self))

    _bass.Bass.to_json_bytes = to_json_bytes
    _bass.Bass._wait_split_installed = True


def install_ntff_shim():
    import sys
    import types

    if "antenv.axon_hooks" in sys.modules:
        return
    mod = types.ModuleType("antenv.axon_hooks")
    state = {"hook": None}

    def set_axon_ntff_profile_hook(h):
        state["hook"] = h

    def get_axon_ntff_profile_hook():
        return state["hook"]

    mod.set_axon_ntff_profile_hook = set_axon_ntff_profile_hook
    mod.get_axon_ntff_profile_hook = get_axon_ntff_profile_hook
    sys.modules["antenv.axon_hooks"] = mod

    try:
        from trn_agent_boot.trn_boot import _ntff_profile_via_ctypes

        hook = _ntff_profile_via_ctypes("/opt/axon/libaxon_pjrt.so")
        if hook is not None:
            set_axon_ntff_profile_hook(hook)
    except Exception as e:
        print(f"ntff shim: hook install failed: {e}")

    from concourse import bass_utils

    bass_utils.upload_artifacts = lambda tmpdir: tmpdir


# geometry
B, C, H, W, K = 4, 64, 352, 1216, 3
HALF = 176  # rows per core
STRIP = 88  # rows per strip (2 strips per core in partition halves)
WP = W + 2  # padded width
BLK_ROWS = 16  # input rows per slab block (4 quads)
NBLK = 6  # blocks per strip (96 rows: local input rows -1..94)
NQ = 23  # conv quads per strip (input rows -1..90)
NFQ = 22  # fold quads per strip (out rows 0..87)
XCH = [(0, 512), (512, 512), (1024, 192)]
NCH = len(XCH)
MROW = [0, 1, 2, 3, 8, 4, 5, 6, 7]  # T-row m for output plane p

F32 = mybir.dt.float32
F16 = mybir.dt.float16


def act_reciprocal(nc, out, in_):
    """ACT Reciprocal, bypassing the bass advisory accuracy guard (our
    tolerance is 2e-2; the r8 output is fp16 anyway)."""
    eng = nc.scalar
    inputs = [eng.lower_ap(in_)]
    for arg in (0.0, 1.0, 0.0):  # bias, scale, alpha
        inputs.append(mybir.ImmediateValue(dtype=mybir.dt.float32, value=arg))
    return eng.add_instruction(
        mybir.InstActivation(
            name=nc.get_next_instruction_name(),
            func=mybir.ActivationFunctionType.Reciprocal,
            ins=inputs,
            outs=[eng.lower_ap(out)],
        )
    )


def build_nc():
    nc = bass.Bass()
    slab = nc.dram_tensor(
        "slab", [128, NBLK, BLK_ROWS * WP], F16, kind="ExternalInput"
    )
    w27 = nc.dram_tensor("w27", [128, 162], F16, kind="ExternalInput")
    fsel = nc.dram_tensor("fsel", [128, 54], F16, kind="ExternalInput")
    rbsel = nc.dram_tensor("rbsel", [128, 288], F16, kind="ExternalInput")
    ssel = nc.dram_tensor("ssel", [128, 8], F16, kind="ExternalInput")
    bias = nc.dram_tensor("bias", [128, 1], F32, kind="ExternalInput")
    out = nc.dram_tensor("out", [NFQ // 2, 128, 2 * W], F16, kind="ExternalOutput")

    with TileContext(nc) as tc:
        with (
            tc.tile_pool(name="consts", bufs=1) as cpool,
            tc.tile_pool(name="slabp", bufs=2) as slabp,
            tc.tile_pool(name="epool", bufs=16) as epool,
            tc.tile_pool(name="ysp", bufs=14) as ysp,
            tc.tile_pool(name="aasp", bufs=3) as aasp,
            tc.tile_pool(name="r8p", bufs=4) as r8p,
            tc.tile_pool(name="r32p", bufs=2) as r32p,
            tc.tile_pool(name="stagep", bufs=2) as stagep,
            tc.tile_pool(name="pconv", bufs=3, space="PSUM") as pconv,
            tc.tile_pool(name="pfold", bufs=2, space="PSUM") as pfold,
            tc.tile_pool(name="ps_s", bufs=2, space="PSUM") as ps_s,
            tc.tile_pool(name="ps_rbc", bufs=1, space="PSUM") as ps_rbc,
        ):
            w27t = cpool.tile([128, 162], F16, name="w27t")
            fselt = cpool.tile([128, 54], F16, name="fselt")
            rbselt = cpool.tile([128, 288], F16, name="rbselt")
            sselt = cpool.tile([128, 8], F16, name="sselt")
            biast = cpool.tile([128, 1], F32, name="biast")

            # first slab block split: cols for chunk 0 first, then the rest
            st0 = slabp.tile([128, BLK_ROWS * WP], F16, name="st")
            st0v = st0[:].rearrange("p (r w) -> p r w", r=BLK_ROWS, w=WP)
            sl0v = slab[:, 0, :].rearrange("p (r w) -> p r w", r=BLK_ROWS, w=WP)
            # first the weights + the rows for the first two quads, so the
            # conv starts ~4us in; trailing rows and other consts follow
            nc.sync.dma_start(w27t[:], w27[:])
            nc.sync.dma_start(st0v[:, 0:8, :], sl0v[:, 0:8, :])
            nc.gpsimd.dma_start(biast[:], bias[:])
            nc.gpsimd.dma_start(st0v[:, 8:BLK_ROWS, :], sl0v[:, 8:BLK_ROWS, :])
            nc.gpsimd.dma_start(fselt[:], fsel[:])
            nc.gpsimd.dma_start(rbselt[:], rbsel[:])
            nc.gpsimd.dma_start(sselt[:], ssel[:])

            # one-time PSUM inits (banks rotate within fixed pools):
            # conv + fold banks to 0 (pad rows must read as 0);
            # S banks to 1.0 (reciprocal must not see garbage).
            for pool, nm, cnt in ((pconv, "cv", 3), (pfold, "pf", 2), (ps_rbc, "rbc", 1)):
                for _ in range(cnt):
                    tz = pool.tile([128, 512], F32, name=nm)
                    nc.vector.memset(tz[:], 0.0)
            for _ in range(2):
                tz = ps_s.tile([128, 512], F32, name="s8")
                nc.vector.memset(tz[:], 1.0)

            # ---- software pipeline state ----
            etiles = {}  # (s, j, ch) -> E tile
            recip_batch = {"tile": None, "members": [], "count": 0}
            post_recip = []  # fold-units waiting on their batch reciprocal
            pending_smm = []  # fold-units whose S matmul is deferred 1 unit
            stage_tiles = {}  # (group, ) -> stage tile

            def do_recip_flush():
                b = recip_batch
                if not b["members"]:
                    return
                r8s = [r8p.tile([128, 512], F16, name="r8") for _ in range(2)]
                with nc.allow_low_precision(reason="fp16 normalize"):
                    act_reciprocal(nc, r8s[0][:, 0:512], b["tiles"][0][:, 0:512])
                    if b["tiles"][1] is not None:
                        act_reciprocal(nc, r8s[1][:, 0:512], b["tiles"][1][:, 0:512])
                for u, fu in enumerate(b["members"]):
                    fu["r8"] = r8s[u // 4]
                    fu["u"] = u % 4
                b["tiles"] = [None, None]
                b["members"] = []

            def do_smm(fu):
                b = recip_batch
                u = len(b["members"])
                if u % 4 == 0:
                    b.setdefault("tiles", [None, None])[u // 4] = ps_s.tile(
                        [128, 512], F32, name="s8"
                    )
                n = fu["n"]
                nc.tensor.matmul(
                    out=b["tiles"][u // 4][32 * (u % 4) : 32 * (u % 4) + 8, 0:n],
                    lhsT=sselt[0:114, 0:8],
                    rhs=fu["aas"][0:114, 0:n],
                    start=True,
                    stop=True,
                    tile_position=(0, 32 * (u % 4)),
                    skip_group_check=True,
                )
                b["members"].append(fu)
                if len(b["members"]) == 8:
                    do_recip_flush()

            def do_post(fu):
                # rbc broadcast matmuls + normalize multiply + stage DMA
                n, x0, ch, fj = fu["n"], fu["x0"], fu["ch"], fu["fj"]
                u = fu["u"]
                rbc = ps_rbc.tile([128, 512], F32, name="rbc")
                for q in range(4):
                    bidx = 4 * u + q
                    nc.tensor.matmul(
                        out=rbc[32 * q : 32 * q + 18, 0:n],
                        lhsT=rbselt[:, 18 * bidx : 18 * bidx + 18],
                        rhs=fu["r8"][:, 0:n],
                        start=True,
                        stop=True,
                        tile_position=(0, 32 * q),
                        skip_group_check=True,
                    )
                g, k2 = fj // 2, fj % 2
                if (g, ) not in stage_tiles:
                    stage_tiles[(g, )] = stagep.tile(
                        [128, 2 * W], F16, name="stage"
                    )
                stage = stage_tiles[(g, )]
                sv = stage[:].rearrange("p (k x) -> p k x", k=2, x=W)
                with nc.allow_low_precision(reason="fp16 output staging"):
                    nc.vector.tensor_mul(
                        sv[0:114, k2, x0 : x0 + n],
                        fu["ys"][0:114, 0:n],
                        rbc[0:114, 0:n],
                    )
                if k2 == 1 and ch == NCH - 1:
                    # group complete: one dense raw dump (host de-interleaves)
                    nc.sync.dma_start(out[g], stage[:])
                    del stage_tiles[(g, )]

            def drain_posts(limit):
                k = 0
                while post_recip and "r8" in post_recip[0] and k < limit:
                    do_post(post_recip.pop(0))
                    k += 1

            def do_fold(fj, ch, n, x0):
                # fold quad fj: K=128 untiled matmuls on per-strip E tiles
                # (bands at 32c + 9di + m); pair0 = pattern A on E_j,
                # pair1 = pattern B on E_j + pattern C on E_{j+1}
                pf = pfold.tile([128, 512], F32, name="pf")
                es = [(etiles[(s, fj, ch)], etiles[(s, fj + 1, ch)]) for s in range(2)]
                for s in range(2):
                    nc.tensor.matmul(
                        out=pf[64 * s : 64 * s + 18, 0:n],
                        lhsT=fselt[:, 0:18],
                        rhs=es[s][0][:, 0:n],
                        start=True,
                        stop=True,
                        tile_position=(0, 64 * s),
                        skip_group_check=True,
                    )
                for s in range(2):
                    nc.tensor.matmul(
                        out=pf[64 * s + 32 : 64 * s + 50, 0:n],
                        lhsT=fselt[:, 18:36],
                        rhs=es[s][0][:, 0:n],
                        start=True,
                        stop=False,
                        tile_position=(0, 64 * s + 32),
                        skip_group_check=True,
                    )
                for s in range(2):
                    nc.tensor.matmul(
                        out=pf[64 * s + 32 : 64 * s + 50, 0:n],
                        lhsT=fselt[:, 36:54],
                        rhs=es[s][1][:, 0:n],
                        start=False,
                        stop=True,
                        tile_position=(0, 64 * s + 32),
                        skip_group_check=True,
                    )
                if ch == NCH - 1:
                    # E tiles of quad fj fully consumed
                    for s in range(2):
                        for c2 in range(NCH):
                            etiles.pop((s, fj, c2), None)
                import os
                ys = ysp.tile([128, 512], F16, name="ys")
                aas = aasp.tile([128, 512], F16, name="aas")
                _v = os.environ.get("CSPN_VARIANT", "")
                with nc.allow_low_precision(reason="fp16 y staging"):
                    nc.vector.tensor_copy(ys[0:114, 0:n], pf[0:114, 0:n])
                    # abs on DVE from the fp16 SBUF copy: max(-ys, ys)
                    nc.vector.scalar_tensor_tensor(
                        out=aas[0:114, 0:n],
                        in0=ys[0:114, 0:n],
                        scalar=-1.0,
                        in1=ys[0:114, 0:n],
                        op0=mybir.AluOpType.mult,
                        op1=mybir.AluOpType.max,
                    )
                fu = {"n": n, "x0": x0, "ch": ch, "fj": fj, "ys": ys, "aas": aas}
                # defer this funit's S matmul one unit so the PE never waits
                # on the DVE abs chain
                pending_smm.append(fu)
                if len(pending_smm) > 1:
                    fu2 = pending_smm.pop(0)
                    do_smm(fu2)
                    post_recip.append(fu2)
                drain_posts(1)

            # ---- main loop: conv quads ----
            st = st0
            st_next = None
            for j in range(NQ):
                blk = (4 * j) // BLK_ROWS
                if j % 4 == 0 and blk + 1 < NBLK:
                    st_next = slabp.tile([128, BLK_ROWS * WP], F16, name="st")
                    nc.gpsimd.dma_start(st_next[:], slab[:, blk + 1, :])
                stv = st[:].rearrange("p (r w) -> p r w", r=BLK_ROWS, w=WP)
                for ch, (x0, n) in enumerate(XCH):
                    cbs = [
                        pconv.tile([128, 512], F32, name="cv"),
                        pconv.tile([128, 512], F32, name="cv"),
                    ]
                    # uniform un-row-tiled 128x32 mode everywhere (mode
                    # switches drain the PE); c innermost so consecutive MMs
                    # land on different col tiles; the wrong strip's rows are
                    # zeroed in the weights. Last quad: rows 89,90 are
                    # garbage (never read by any fold) - skip their matmuls.
                    cmax = 2 if j == NQ - 1 else 4
                    for dj in range(3):
                        for s in range(2):
                            for c in range(cmax):
                                rib = (4 * j - 1 + c) - BLK_ROWS * blk + 1
                                nc.tensor.matmul(
                                    out=cbs[s][32 * c : 32 * c + 27, 0:n],
                                    lhsT=w27t[
                                        :,
                                        27 * (3 * s + dj) : 27 * (3 * s + dj) + 27,
                                    ],
                                    rhs=stv[
                                        :,
                                        rib,
                                        x0 + dj : x0 + dj + n,
                                    ],
                                    start=(dj == 0),
                                    stop=(dj == 2),
                                    tile_position=(0, 32 * c),
                                    skip_group_check=True,
                                )
                    # evacuate conv psum (+bias/3 on each band) to E tiles
                    with nc.allow_low_precision(reason="fp16 band staging"):
                        for s in range(2):
                            et = epool.tile([128, 512], F16, name="et")
                            nc.scalar.activation(
                                et[:, 0:n],
                                cbs[s][:, 0:n],
                                mybir.ActivationFunctionType.Identity,
                                bias=biast[:, 0:1],
                                scale=1.0,
                            )
                            etiles[(s, j, ch)] = et
                    # fold of quad j-2 after the evac emission (scalar queue
                    # gets the evacs first); its E tiles are 3+ units old
                    if j >= 2:
                        do_fold(j - 2, ch, n, x0)
                if (4 * (j + 1)) // BLK_ROWS != blk and blk + 1 < NBLK:
                    st = st_next
            # epilogue: last fold quad + drain
            for ch, (x0, n) in enumerate(XCH):
                do_fold(NFQ - 1, ch, n, x0)
            while pending_smm:
                fu2 = pending_smm.pop(0)
                do_smm(fu2)
                post_recip.append(fu2)
            do_recip_flush()
            drain_posts(len(post_recip) + 1)
    return nc


def make_consts(conv_w, gamma, beta, mean, var):
    eps = 1e-5
    s = gamma.astype(np.float64) / np.sqrt(var.astype(np.float64) + eps)
    bt = beta.astype(np.float64) - mean.astype(np.float64) * s
    wp = conv_w.astype(np.float64) * s[:, None, None, None]  # [8, 64, 3, 3]
    bt9 = np.concatenate([bt, [bt.sum()]])  # [9]

    # w27 [128, 162]: col block (s, dj) at 27*(3s+dj); rows 64s..64s+64 hold
    # the weights (cols 9*di + m within block), the other strip's rows are 0
    w27 = np.zeros((128, 162), np.float32)
    for s in range(2):
        for dj in range(3):
            for di in range(3):
                blk = wp[:, :, di, dj]  # [oc, c]
                col = 27 * (3 * s + dj) + 9 * di
                w27[64 * s : 64 * s + 64, col : col + 8] = blk.T
                w27[64 * s : 64 * s + 64, col + 8] = blk.sum(axis=0)

    # bias [128, 1]: rows 32c + 9di + m -> bt9[m]/3
    bias = np.zeros((128, 1), np.float32)
    for c in range(4):
        for di in range(3):
            bias[32 * c + 9 * di : 32 * c + 9 * di + 9, 0] = bt9 / 3.0

    # fold selectors [128, 36]: patterns P1, P2 (M=18, col 2m+rp),
    # identical in both partition halves. E-tile local rows: first band at
    # 9di+m base 0 (ev: c0 / od: c2), second at 32+9di+m (ev: c1 / od: c3).
    # P1: rp0 <- (band0, di0) rows m, (band1, di1) rows 41+m;
    #     rp1 <- (band1, di0) rows 32+m.
    # P2: rp0 <- (band0, di2) rows 18+m;
    #     rp1 <- (band0, di1) rows 9+m, (band1, di2) rows 50+m.
    fsel = np.zeros((128, 54), np.float32)

    def fs(pat, rp, m, c, di):
        fsel[32 * c + 9 * di + m, 18 * pat + 2 * m + rp] = 1.0

    for m in range(9):
        for di in range(3):
            fs(0, 0, m, di, di)
            fs(0, 1, m, 1 + di, di)
        fs(1, 0, m, 2, 0)
        fs(1, 0, m, 3, 1)
        fs(1, 1, m, 3, 0)
        fs(2, 0, m, 0, 2)
        fs(2, 1, m, 0, 1)
        fs(2, 1, m, 1, 2)

    # ssel [128, 8]: row 32q + 2m + rp -> col 2q + rp (m = 0..7)
    ssel = np.zeros((128, 8), np.float32)
    for q in range(4):
        for rp in range(2):
            for m in range(8):
                ssel[32 * q + 2 * m + rp, 2 * q + rp] = 1.0

    # rbsel [128, 288]: 16 patterns (u batch slot, q): col 18*(4u+q) + 2m+rp,
    # nonzero at r8{A,B} row 32u + 2*(q%2) + rp; -1 for m=8 (T row)
    rbsel = np.zeros((128, 288), np.float32)
    for u in range(4):
        for q in range(4):
            for rp in range(2):
                for m in range(9):
                    rbsel[32 * u + 2 * q + rp, 18 * (4 * u + q) + 2 * m + rp] = (
                        -1.0 if m == 8 else 1.0
                    )

    return (
        w27.astype(np.float16),
        fsel.astype(np.float16),
        rbsel.astype(np.float16),
        ssel.astype(np.float16),
        bias,
    )


TRACE = False
LAST_EXEC_NS = None


def kernel(feature, conv_w, gamma, beta, mean, var, kernel_size):
    global LAST_EXEC_NS
    install()
    if TRACE:
        install_ntff_shim()

    from concourse.bass_utils import run_bass_kernel_spmd

    feature = np.asarray(feature, np.float32)
    conv_w = np.asarray(conv_w, np.float32)
    gamma = np.asarray(gamma, np.float32)
    beta = np.asarray(beta, np.float32)
    mean = np.asarray(mean, np.float32)
    var = np.asarray(var, np.float32)

    w27, fsel, rbsel, ssel, bias = make_consts(conv_w, gamma, beta, mean, var)

    # padded feature with extra tail rows for the slab block overhang
    fpad = np.zeros((B, C, H + 10, WP), np.float32)
    fpad[:, :, 1 : H + 1, 1 : W + 1] = feature
    fpad16 = fpad.astype(np.float16)

    in_maps = []
    for core in range(8):
        b, half = core // 2, core % 2
        h0 = half * HALF
        # strip A: core rows 0..87  -> fpad rows h0 + [0, 96)   (block k: 16k)
        # strip B: core rows 88..175 -> fpad rows h0+88 + [0, 96)
        slab2 = np.empty((128, NBLK, BLK_ROWS * WP), np.float16)
        for k in range(NBLK):
            r0 = h0 + BLK_ROWS * k
            slab2[0:64, k, :] = fpad16[b, :, r0 : r0 + BLK_ROWS, :].reshape(C, -1)
            slab2[64:128, k, :] = fpad16[
                b, :, r0 + STRIP : r0 + STRIP + BLK_ROWS, :
            ].reshape(C, -1)
        in_maps.append(
            {
                "slab": slab2,
                "w27": w27,
                "fsel": fsel,
                "rbsel": rbsel,
                "ssel": ssel,
                "bias": bias,
            }
        )

    nc = build_nc()
    res = run_bass_kernel_spmd(nc, in_maps, core_ids=list(range(8)), trace=TRACE)
    LAST_EXEC_NS = res.exec_time_ns

    out_full = np.zeros((B, 9, H + 2, WP), np.float32)
    for core in range(8):
        b, half = core // 2, core % 2
        h0 = half * HALF
        raw = res.results[core]["out"]  # [11, 128, 2*W] fp16
        # row 32q + 2m + rp (q = 2s + pp), free (k, x);
        # out row y = s*88 + 8g + 4k + 2pp + rp
        v = raw.reshape(NFQ // 2, 2, 2, 32, 2, W)  # g s pp v k x
        for p in range(9):
            i, j = p // 3, p % 3
            m = MROW[p]
            sub = v[:, :, :, 2 * m : 2 * m + 2, :, :]  # g s pp rp k x
            plane = (
                sub.transpose(1, 0, 4, 2, 3, 5)  # s g k pp rp x
                .reshape(HALF, W)
                .astype(np.float32)
            )
            if p == 4:
                plane = 1.0 + plane
            out_full[b, p, h0 + i : h0 + HALF + i, j : j + W] = plane
    return out_full
